# revision 3
# baseline (speedup 1.0000x reference)
"""Trainium2 Bass kernel for nn_KeyedConv2d: 3x3 SAME conv, stride 1.

x: [8, 64, 64, 64] (NCHW), Wt: [64, 64, 3, 3] (OIHW) -> out [8, 64, 64, 64].
Data-parallel over batch, one image per NeuronCore (8 cores).

v6: variable-size chunks.  c0 = 10 rows (640px) so its longer matmul run
absorbs the weights-DMA latency and chunk1's piece arrives in time; c1-c6
= 8 rows; c7 = 6 rows split 4+2 so only a tiny eviction trails the last
matmul.  One merged trailing store (c6+c7a+c7b).  Weights on SWDGE landing
between p0 and p1; pieces p0(0-10), p1(11-18), p2(17-26), p3(25-34),
p45(33-50), p67(49-63).  Warmups read a raw sbuf tensor (no memset dep)
so pe_busy_start pins at ~750ns and all real matmuls run full-clock.
"""
import numpy as np
import ml_dtypes

import concourse.bass as bass
import concourse.mybir as mybir
import concourse.tile as tile
from concourse import bacc
from concourse.bass_utils import run_bass_kernel_spmd

F32 = mybir.dt.float32
BF16 = mybir.dt.bfloat16

IC = OC = 64
H = W = 64
K = 3
HWPIX = H * W        # 4096
PW = W + 1           # padded row width (left zero col)

WARMN = 2            # warmup matmuls (set pe_busy_start early)

# chunk map: (first out row, n out rows); strip k serves chunk k
CH_R0 = [0, 10, 18, 26, 34, 42, 50, 58]
CH_NR = [10, 8, 8, 8, 8, 8, 8, 6]
NCH = len(CH_R0)
# chunk 7 is computed as two sub-chunks (rbase, nrows) within strip 7
C7_SPLIT = [(0, 4), (4, 2)]
# osb column offset per store region (c0..c6, c7a, c7b)
OFFS = np.cumsum([0] + [nr * W // 2 for nr in CH_NR[:-1]]
                 + [C7_SPLIT[0][1] * W // 2]).tolist()  # 9 entries

# pieces: (first img row, last img row); strip k -> piece PIECE_OF[k]
PIECE_ROWS = [(0, 10), (11, 18), (17, 26), (25, 34), (33, 42), (41, 50),
              (49, 58), (57, 63)]
PIECE_OF = [0, None, 2, 3, 4, 5, 6, 7]   # s1 is split p0/p1

MODE = "v6"


def _build(mode: str = MODE) -> bacc.Bacc:
    nc = bacc.Bacc("TRN2", target_bir_lowering=False, debug=False)

    xbf = nc.dram_tensor("xbf", [IC, H, W], BF16, kind="ExternalInput").ap()
    wt = nc.dram_tensor("wt", [128, 6 * 2 * OC], BF16, kind="ExternalInput").ap()
    y = nc.dram_tensor("y", [128, HWPIX // 2], BF16, kind="ExternalOutput").ap()

    with tile.TileContext(nc) as tc:
        with (
            tc.tile_pool(name="wsb", bufs=1) as wsb_pool,
            tc.tile_pool(name="piece", bufs=1) as piece_pool,
            tc.tile_pool(name="xs", bufs=1) as xs_pool,
            tc.tile_pool(name="psum", bufs=1, space="PSUM") as psum_pool,
            tc.tile_pool(name="osb", bufs=1) as osb_pool,
        ):
            # --- warmups on a raw (untracked, uninitialized) sbuf tensor;
            # results land in ps0 which chunk0 overwrites with start=True
            warm = nc.alloc_sbuf_tensor("warm_raw", [64, 128], BF16).ap()
            wps = psum_pool.tile([128, CH_NR[0] * W // 2], F32, name="ps0")
            for _ in range(WARMN):
                nc.tensor.matmul(
                    wps[0:64, 0:128], warm[:, 0:64], warm[:, 0:128],
                    start=True, stop=True, skip_group_check=True,
                )

            # --- weights via SWDGE (gpsimd): desc-gen ~680-1720, transfer
            # slots in right after piece0 on the (serial) DMA engines
            wsb = wsb_pool.tile([128, 6 * 2 * OC], BF16)
            nc.gpsimd.dma_start(wsb[:, :], wt)

            # --- output staging
            osb = osb_pool.tile([128, HWPIX // 2], BF16)

            # --- staging pieces (HWDGE/SP)
            pieces = {}
            prows = {}
            for pi, (r0, r1) in enumerate(PIECE_ROWS):
                nr = r1 - r0 + 1
                pc = piece_pool.tile([128, nr * W], BF16, name=f"pc{pi}")
                nc.sync.dma_start(
                    pc[:, :], xbf[:, r0:r1 + 1, :].partition_broadcast(2)
                )
                pieces[pi] = pc
                prows[pi] = (r0, nr)

            # --- strip tiles + zero slivers
            # memsets: strips 0-4 DVE (early), strips 5-7 Pool
            xss = []
            srows = []
            for k in range(NCH):
                sr = CH_NR[k] + 2          # padded rows incl halo
                tlen = sr * PW + 2
                xs = xs_pool.tile([128, tlen], BF16, name=f"xs{k}")
                xss.append(xs)
                srows.append(sr)
                eng = nc.vector if k <= 4 else nc.gpsimd
                eng.memset(
                    xs[0:64, 0:sr * PW].rearrange(
                        "p (a b) -> p a b", b=PW)[:, :, 0:1],
                    0.0,
                )
                eng.memset(xs[0:64, sr * PW:sr * PW + 1], 0.0)
                eng.memset(
                    xs[64:128, 0:sr * PW].rearrange(
                        "p (a b) -> p a b", b=PW)[:, :, W:PW],
                    0.0,
                )
                if k == 0:
                    eng.memset(xs[:, 0:PW], 0.0)              # pad row 0
                if k == NCH - 1:
                    eng.memset(
                        xs[:, (sr - 1) * PW:(sr - 1) * PW + PW], 0.0)  # row 64

            # --- pad-copy jobs (strip, piece, strip-rows rlo..rhi, half,
            # engine).  strip row r = img row CH_R0[k]-1+r.  Strips 0-1 all
            # DVE (Act copies scare the scheduler into hoisting later
            # chunks); steady state: DVE top + bottom rows 0-4, Pool bottom
            # rows 5+; Act only evictions.
            T, B = 0, 1
            copy_jobs = [
                (0, 0, 1, 6, T, nc.vector), (0, 0, 1, 6, B, nc.vector),
                (0, 0, 7, 11, T, nc.vector), (0, 0, 7, 11, B, nc.gpsimd),
                (1, 0, 0, 1, T, nc.vector), (1, 0, 0, 1, B, nc.vector),
                (1, 1, 2, 9, T, nc.vector), (1, 1, 2, 9, B, nc.vector),
            ]
            for k in range(2, NCH):
                rh = srows[k] - 1 if k < NCH - 1 else srows[k] - 2
                copy_jobs += [
                    (k, PIECE_OF[k], 0, rh, T, nc.vector),
                    (k, PIECE_OF[k], 0, 4, B, nc.vector),
                    (k, PIECE_OF[k], 5, rh, B, nc.gpsimd),
                ]

            for k, key, rlo, rhi, half, eng in copy_jobs:
                pc = pieces[key]
                xs = xss[k]
                r0, nr = prows[key]
                off = CH_R0[k] + rlo - 1 - r0
                assert 0 <= off and off + (rhi - rlo) < nr, (k, key, rlo, rhi)
                src = pc[:, off * W:(off + rhi - rlo + 1) * W].rearrange(
                    "p (a b) -> p a b", b=W)
                dst = xs[:, rlo * PW:(rhi + 1) * PW].rearrange(
                    "p (a b) -> p a b", b=PW)
                if half == T:
                    eng.tensor_copy(dst[0:64, :, 1:1 + W], src[0:64, :, :])
                else:
                    eng.tensor_copy(dst[64:128, :, 0:W], src[64:128, :, :])

            # --- conv chunks.  work: (strip, rbase, npix, tag, osb off)
            work = []
            for k in range(NCH - 1):
                work.append((k, 0, CH_NR[k] * W, f"ps{k}", OFFS[k]))
            # c7a/c7b reuse c1/c2's PSUM banks (evicted long before);
            # 9 tiles would need 9 banks and PSUM has 8
            work.append((7, C7_SPLIT[0][0], C7_SPLIT[0][1] * W, "ps1", OFFS[7]))
            work.append((7, C7_SPLIT[1][0], C7_SPLIT[1][1] * W, "ps2", OFFS[8]))

            for k, rbase, npix, tag, goff in work:
                xs = xss[k]
                nrows = npix // W
                ps = psum_pool.tile([128, npix // 2], F32, name=tag)
                t = 0
                for ky in range(K):
                    for b0 in (0, 2):
                        bb = (rbase + ky) * PW + b0
                        rhs = xs[:, bb:bb + nrows * PW].rearrange(
                            "p (a b) -> p a b", b=PW)[:, :, 0:W].rearrange(
                            "p a (c t) -> p a c t", t=2)[:, :, :, 0:1]
                        m = ky * 2 + (b0 // 2)
                        nc.tensor.matmul(
                            ps[:, :], wsb[:, m * 128:(m + 1) * 128], rhs,
                            start=(t == 0), stop=(t == 5),
                            skip_group_check=True,
                        )
                        t += 1
                odst = osb[:, goff:goff + npix // 2]
                if tag == "ps6" or (tag == "ps2" and k == 7):
                    nc.vector.tensor_copy(odst, ps[:, :])
                else:
                    nc.scalar.copy(odst, ps[:, :])
                # store flushes: after c1, c3, c5 evictions; trailing store
                # (c6+c7a+c7b) after the last eviction
                if k < 7 and tag in ("ps1", "ps3", "ps5"):
                    g0 = OFFS[k - 1]
                    g1 = OFFS[k + 1]
                    nc.sync.dma_start(y[:, g0:g1], osb[:, g0:g1])
            nc.sync.dma_start(y[:, OFFS[6]:], osb[:, OFFS[6]:])

    nc.compile()
    return nc


_NC_CACHE: dict[str, bacc.Bacc] = {}


def _pack_weights(Wt: np.ndarray) -> np.ndarray:
    """Paired-pixel weight packing: block m = ky*2 + (b0//2),
    lhsT[(ic, s), 64u + oc] = W[oc, ic, ky, kx] for kx = b0 + s - u."""
    Wf = Wt.astype(np.float32)
    wsb = np.zeros((128, 6 * 2 * OC), dtype=np.float32)
    for ky in range(K):
        for b0 in (0, 2):
            m = ky * 2 + (b0 // 2)
            for u in (0, 1):
                for s in (0, 1):
                    kx = b0 + s - u
                    if 0 <= kx < K:
                        wsb[64 * s:64 * s + 64,
                            m * 128 + 64 * u:m * 128 + 64 * u + 64] = (
                            Wf[:, :, ky, kx].T
                        )
    return wsb.astype(ml_dtypes.bfloat16)


def _col_index() -> np.ndarray:
    """colidx[R, c] = y column holding out[:, R, 2c+u] (for both u)."""
    reg_r0 = CH_R0[:-1] + [CH_R0[7], CH_R0[7] + C7_SPLIT[1][0]]
    reg_nr = CH_NR[:-1] + [C7_SPLIT[0][1], C7_SPLIT[1][1]]
    colidx = np.zeros((H, W // 2), np.int32)
    for g in range(len(reg_r0)):
        for r in range(reg_nr[g]):
            colidx[reg_r0[g] + r] = OFFS[g] + r * (W // 2) + np.arange(W // 2)
    return colidx


_COLIDX = _col_index()


def kernel(x: np.ndarray, Wt: np.ndarray) -> np.ndarray:
    assert x.shape == (8, IC, H, W) and Wt.shape == (OC, IC, K, K)
    if MODE not in _NC_CACHE:
        _NC_CACHE[MODE] = _build(MODE)
    nc = _NC_CACHE[MODE]

    wt_t = _pack_weights(Wt)
    in_maps = [
        {
            "xbf": np.ascontiguousarray(x[b].astype(ml_dtypes.bfloat16)),
            "wt": wt_t,
        }
        for b in range(8)
    ]
    global _last_in_maps
    _last_in_maps = in_maps
    res = run_bass_kernel_spmd(nc, in_maps, core_ids=list(range(8)))
    outs = []
    for r in res.results:
        yv = np.asarray(r["y"]).reshape(2, OC, HWPIX // 2)
        # out[oc, R, 2c+u] = yv[u, oc, colidx[R, c]]
        g = yv[:, :, _COLIDX]            # [2, OC, H, W//2]
        outs.append(g.transpose(1, 2, 3, 0).reshape(OC, H, W))
    return np.stack(outs).astype(np.float32)


_last_in_maps: list[dict[str, np.ndarray]] = []
